# revision 19
# baseline (speedup 1.0000x reference)
"""Trainium2 Bass kernel for nn_AttentionModuleEx1 (LKA-style attention module).

Per-sample computation (512 ch, 64x64 spatial):
  attn = dw5x5(x) + b0
  a_i  = dwH(dwW(attn)) dilated separable branches (k=7,11,21, dil=3)
  s    = attn + a0 + a1 + a2
  y    = (W3 @ s + b3) * x        (1x1 pointwise conv over channels)

Sharding: pure data-parallel — batch 8 -> 1 sample per NeuronCore.

Implementation (fp16 compute, fp32 PSUM accumulation):
  - channels on partitions (4 blocks of 128), spatial on the free dim;
    zero-padded SBUF canvases make every conv tap a shifted-AP read.
    Canvases are double-buffered so consecutive channel blocks overlap
    across engines.
  - PE lane: 5x5, W11, W21, H7, H11 conv taps run as diagonal matmuls
    (lhsT = diag(w_tap), host-built) accumulating in PSUM; ScalarE copies
    PSUM->SBUF with the conv bias fused (Identity activation).
  - DVE lane: W7 and H21 taps as fused MACs (scalar_tensor_tensor with
    per-partition scalar weights).
  - pointwise conv: PE matmuls, lhsT = W3^T tiles (transposed on host);
    bias-add + multiply-by-x fused into one scalar_tensor_tensor per chunk.
"""

import sys

for p in ("/opt/trn_rl_repo", "/opt/pypackages"):
    if p not in sys.path:
        sys.path.insert(0, p)

import os

import numpy as np

C, H, W = 512, 64, 64
# taps of the k=21 H-conv / k=7 W-conv moved from the DVE lane to the PE lane
H21_PE = int(os.environ.get("H21_PE", "0"))
W7_PE = int(os.environ.get("W7_PE", "0"))
NBLK = 4  # channel blocks of 128
P = 128

_NC = None


def _build_nc():
    import concourse.bass as bass  # noqa: F401
    import concourse.bacc as bacc
    import concourse.mybir as mybir
    from concourse.tile import TileContext

    f32 = mybir.dt.float32
    f16 = mybir.dt.float16
    A = mybir.AluOpType
    AF = mybir.ActivationFunctionType

    nc = bacc.Bacc(None, target_bir_lowering=False)

    x_d = nc.dram_tensor("x", [C, H, W], f16, kind="ExternalInput")
    b0_d = nc.dram_tensor("b0", [C, 1], f32, kind="ExternalInput")
    w0_1_d = nc.dram_tensor("w0_1", [C, 7], f32, kind="ExternalInput")
    w2_2_d = nc.dram_tensor("w2_2", [C, 21], f32, kind="ExternalInput")
    bsumh_d = nc.dram_tensor("bsumh", [C, 1], f32, kind="ExternalInput")
    # host-built diagonal weight stacks for the PE lane, f16:
    wd5_d = nc.dram_tensor("wd5", [NBLK, 25, P, P], f16, kind="ExternalInput")
    wd11w_d = nc.dram_tensor("wd11w", [NBLK, 11, P, P], f16, kind="ExternalInput")
    wd21w_d = nc.dram_tensor("wd21w", [NBLK, 21, P, P], f16, kind="ExternalInput")
    wd7h_d = nc.dram_tensor("wd7h", [NBLK, 7, P, P], f16, kind="ExternalInput")
    wd11h_d = nc.dram_tensor("wd11h", [NBLK, 11, P, P], f16, kind="ExternalInput")
    wd21h_d = (nc.dram_tensor("wd21h", [NBLK, 21, P, P], f16,
                              kind="ExternalInput") if H21_PE else None)
    wd7w_d = (nc.dram_tensor("wd7w", [NBLK, 7, P, P], f16,
                             kind="ExternalInput") if W7_PE else None)
    w3_d = nc.dram_tensor("w3", [C, C], f16, kind="ExternalInput")  # W3^T (host)
    b3_d = nc.dram_tensor("b3", [C, 1], f32, kind="ExternalInput")
    out_d = nc.dram_tensor("out", [C, H, W], f16, kind="ExternalOutput")

    with TileContext(nc) as tc:
        with tc.tile_pool(name="main", bufs=1) as MP, \
             tc.tile_pool(name="canv", bufs=2) as CP, \
             tc.tile_pool(name="psum", bufs=2, space="PSUM") as PP, \
             tc.tile_pool(name="diag", bufs=16) as DP, \
             tc.tile_pool(name="stage", bufs=4) as SP:

            accs = [MP.tile([P, H * W], f16, tag=f"acc{b}", name=f"acc{b}")
                    for b in range(NBLK)]

            # SBUF-resident weights
            w3ts = []  # [k][m] lhsT tiles [cin 128, cout 128]
            for kk in range(NBLK):
                row = []
                for m in range(NBLK):
                    t = MP.tile([P, P], f16, tag=f"w3t{kk}{m}", name=f"w3t{kk}{m}")
                    nc.sync.dma_start(
                        t[:, :], w3_d[kk * P:(kk + 1) * P, m * P:(m + 1) * P])
                    row.append(t)
                w3ts.append(row)

            wtiles = {}
            percol = (("b0", b0_d, 1), ("w0_1", w0_1_d, 7),
                      ("w2_2", w2_2_d, 21), ("bsumh", bsumh_d, 1),
                      ("b3", b3_d, 1))
            for b in range(NBLK):
                sl = slice(b * P, (b + 1) * P)
                for nm, dd, k in percol:
                    t = MP.tile([P, k], f32, tag=f"{nm}_{b}", name=f"{nm}_{b}")
                    nc.sync.dma_start(t[:, :], dd[sl, :])
                    wtiles[(nm, b)] = t

            def pe_conv(b, ps_list, groups, dests, bias_ap):
                """groups: list of (diag_dram, ktaps, rview) accumulated into
                one PSUM group per 2048-chunk; dests: list of per-chunk dest
                AP callables (each gets two 16-row ACT copies, bias fused)."""
                for c in range(2):           # two 2048 chunks (32 rows each)
                    ps = PP.tile([P, 2048], f32, tag="ps", name="ps")
                    first = True
                    ngrp = len(groups)
                    for gi, (dd, kt, rv) in enumerate(groups):
                        for t in range(kt):
                            dt_t = DP.tile([P, P], f16, tag="diag", name="diag")
                            nc.sync.dma_start(dt_t[:, :], dd[b, t])
                            last = (gi == ngrp - 1) and (t == kt - 1)
                            for j in range(4):   # four N=512 matmuls per tap
                                r0 = 32 * c + 8 * j
                                nc.tensor.matmul(
                                    ps[:, 512 * j:512 * (j + 1)],
                                    dt_t[:, :], rv(t, r0, r0 + 8),
                                    start=first, stop=last)
                            first = False
                    ps3 = ps.rearrange("p (a b) -> p a b", a=32)
                    for dest in dests:
                        for half in range(2):
                            nc.scalar.activation(
                                dest(c, half), ps3[:, 16 * half:16 * half + 16, :],
                                AF.Identity, bias=bias_ap, scale=1.0)
                    ps_list.append(ps)

            for b in range(NBLK):
                sl = slice(b * P, (b + 1) * P)
                # double-buffered canvases; zero the pads on first use of
                # each of the two pool slots, interiors are always rewritten
                xcan = CP.tile([P, 68, 68], f16, tag="xcan", name="xcan")
                attncan = CP.tile([P, 64, 124], f16, tag="attncan", name="attncan")
                z7 = CP.tile([P, 82, 64], f16, tag="z7", name="z7")
                z11 = CP.tile([P, 94, 64], f16, tag="z11", name="z11")
                z21 = CP.tile([P, 124, 64], f16, tag="z21", name="z21")
                if b < 2:
                    nc.gpsimd.memset(xcan[:, :, :], 0.0)
                    nc.gpsimd.memset(attncan[:, :, :], 0.0)
                    nc.gpsimd.memset(z7[:, :, :], 0.0)
                    nc.gpsimd.memset(z11[:, :, :], 0.0)
                    nc.gpsimd.memset(z21[:, :, :], 0.0)
                nc.gpsimd.dma_start(xcan[:, 2:66, 2:66], x_d[sl, :, :])
                attn_int = attncan[:, :, 30:94]
                acc3 = accs[b].rearrange("p (a b) -> p a b", a=H)

                # ---- 5x5 depthwise on PE (25 diag matmuls, pad 2);
                #      writes attn interior AND acc (bias b0 fused) ----
                def rv5(t, r0, r1):
                    dh, dw = t // 5, t % 5
                    return xcan[:, dh + r0:dh + r1, dw:dw + 64]

                pe_conv(b, [], [(wd5_d, 25, rv5)],
                        [lambda c, h2: attn_int[:, 32 * c + 16 * h2:
                                                32 * c + 16 * h2 + 16, :]],
                        wtiles[("b0", b)][:, 0:1])

                # ---- W-convs ----
                def rvw(pad):
                    def rv(t, r0, r1):
                        col0 = 30 + 3 * t - pad
                        return attncan[:, r0:r1, col0:col0 + 64]
                    return rv

                def destz(zc, zpad):
                    def dest(c, h2):
                        r = zpad + 32 * c + 16 * h2
                        return zc[:, r:r + 16, :]
                    return dest

                # k=11 and k=21 on PE (into z11 rows 15..79, z21 rows 30..94)
                pe_conv(b, [], [(wd11w_d, 11, rvw(15))],
                        [destz(z11, 15)], 0.0)
                pe_conv(b, [], [(wd21w_d, 21, rvw(30))],
                        [destz(z21, 30)], 0.0)

                # k=7: first W7_PE taps on PE, rest on DVE (STT chain), pad 9
                z7i = z7[:, 9:73, :]
                w1t = wtiles[("w0_1", b)]
                if W7_PE:
                    pe_conv(b, [], [(wd7w_d, W7_PE, rvw(9))],
                            [destz(z7, 9)], 0.0)
                for c in range(2):
                    z7c = z7[:, 9 + 32 * c:9 + 32 * c + 32, :]
                    for t in range(W7_PE, 7):
                        col0 = 30 + 3 * t - 9
                        av3 = attncan[:, 32 * c:32 * c + 32, col0:col0 + 64]
                        if t == 0:
                            nc.vector.tensor_scalar_mul(z7c, av3, w1t[:, 0:1])
                        else:
                            tmp = SP.tile([P, 32, 64], f16, tag="ptmp", bufs=2,
                                          name="ptmp")
                            nc.vector.tensor_scalar_mul(
                                tmp[:, :, :], av3, w1t[:, t:t + 1])
                            nc.vector.tensor_tensor(
                                z7c, tmp[:, :, :], z7c, op=A.add)

                # ---- H-convs ----
                # k=7 + k=11 (+ first H21_PE taps of k=21) fused on PE into
                # one PSUM accumulation
                def rvh(zc):
                    def rv(t, r0, r1):
                        row0 = 3 * t  # zpad + 3t - pad = 3t for all branches
                        return zc[:, row0 + r0:row0 + r1, :]
                    return rv

                hgroups = [(wd7h_d, 7, rvh(z7)), (wd11h_d, 11, rvh(z11))]
                if H21_PE:
                    hgroups.append((wd21h_d, H21_PE, rvh(z21)))
                hsum = SP.tile([P, H * W], f16, tag="hsum", bufs=2, name="hsum")
                hsum3 = hsum.rearrange("p (a b) -> p a b", a=H)
                pe_conv(b, [], hgroups,
                        [lambda c, h2: hsum3[:, 32 * c + 16 * h2:
                                             32 * c + 16 * h2 + 16, :]],
                        wtiles[("bsumh", b)][:, 0:1])

                # k=21 H-conv tail on DVE (STT chain into acc); first tap also
                # initializes acc = attn + w*z
                w2t = wtiles[("w2_2", b)]
                for t in range(H21_PE, 21):
                    zv = z21[:, 3 * t:3 * t + 64, :]
                    nc.vector.scalar_tensor_tensor(
                        acc3, zv, w2t[:, t:t + 1],
                        attn_int if t == H21_PE else acc3, op0=A.mult, op1=A.add)

                # combine: acc += (H7 + H11 + all folded biases)
                for c in range(2):
                    av = accs[b][:, 2048 * c:2048 * (c + 1)]
                    nc.vector.scalar_tensor_tensor(
                        av, hsum[:, 2048 * c:2048 * (c + 1)], 1.0, av,
                        op0=A.mult, op1=A.add)

            # ---- pointwise 1x1 conv + bias + multiply-by-x ----
            NCH = 8  # 512-column chunks of the 4096 spatial dim
            for m in range(NBLK):
                sl = slice(m * P, (m + 1) * P)
                xcan = CP.tile([P, 68, 68], f16, tag="xcan", name="xcan")
                nc.gpsimd.dma_start(xcan[:, 2:66, 2:66], x_d[sl, :, :])
                for nch in range(NCH):
                    ps = PP.tile([P, 512], f32, tag="ps", name="ps")
                    for kk in range(NBLK):
                        nc.tensor.matmul(
                            ps[:, :], w3ts[kk][m][:, :],
                            accs[kk][:, nch * 512:(nch + 1) * 512],
                            start=(kk == 0), stop=(kk == NBLK - 1))
                    yb = SP.tile([P, 8, 64], f16, tag="yb", name="yb")
                    ps3 = ps.rearrange("p (a b) -> p a b", a=8)
                    nc.scalar.activation(yb[:, :, :], ps3, AF.Identity,
                                         bias=wtiles[("b3", m)][:, 0:1],
                                         scale=1.0)
                    ost = SP.tile([P, 8, 64], f16, tag="ost", name="ost")
                    xv = xcan[:, 2 + 8 * nch:2 + 8 * nch + 8, 2:66]
                    nc.vector.tensor_tensor(
                        ost[:, :, :], yb[:, :, :], xv, op=A.mult)
                    nc.sync.dma_start(
                        out_d[sl, 8 * nch:8 * nch + 8, :], ost[:, :, :])

    if not nc.is_finalized():
        nc.finalize()
    return nc


def _get_nc():
    global _NC
    if _NC is None:
        _NC = _build_nc()
    return _NC


def _diag_stack(w):
    """w: (C, k) f32 -> (NBLK, k, P, P) f16 diagonal stacks."""
    k = w.shape[1]
    d = np.zeros((NBLK, k, P, P), np.float16)
    idx = np.arange(P)
    wb = w.reshape(NBLK, P, k).transpose(0, 2, 1).astype(np.float16)  # (4,k,128)
    d[:, :, idx, idx] = wb
    return np.ascontiguousarray(d)


def _prep_inputs(inputs):
    f = lambda a, shp: np.ascontiguousarray(
        np.asarray(a, dtype=np.float32).reshape(shp))
    g = lambda nm, k: f(inputs[nm], (C, k))
    com = {
        "b0": f(inputs["b0"], (C, 1)),
        "w0_1": g("w0_1", 7),
        "w2_2": g("w2_2", 21),
        "bsumh": f(np.asarray(inputs["b0_2"], np.float32)
                   + np.asarray(inputs["b1_2"], np.float32)
                   + np.asarray(inputs["b2_2"], np.float32)
                   + g("w0_2", 7).sum(1) * np.asarray(inputs["b0_1"], np.float32).reshape(C)
                   + g("w1_2", 11).sum(1) * np.asarray(inputs["b1_1"], np.float32).reshape(C)
                   + g("w2_2", 21).sum(1) * np.asarray(inputs["b2_1"], np.float32).reshape(C),
                   (C, 1)),
        "wd5": _diag_stack(g("w0", 25)),
        "wd11w": _diag_stack(g("w1_1", 11)),
        "wd21w": _diag_stack(g("w2_1", 21)),
        "wd7h": _diag_stack(g("w0_2", 7)),
        "wd11h": _diag_stack(g("w1_2", 11)),
        **({"wd21h": _diag_stack(g("w2_2", 21))} if H21_PE else {}),
        **({"wd7w": _diag_stack(g("w0_1", 7))} if W7_PE else {}),
        "w3": np.ascontiguousarray(
            np.asarray(inputs["w3"], np.float32).reshape(C, C).T
            .astype(np.float16)),
        "b3": f(inputs["b3"], (C, 1)),
    }
    x = np.asarray(inputs["x"], np.float32).astype(np.float16)
    return [dict(com, x=np.ascontiguousarray(x[i])) for i in range(x.shape[0])]


def run(inputs, trace=False):
    from concourse.bass_utils import run_bass_kernel_spmd
    nc = _get_nc()
    in_maps = _prep_inputs(inputs)
    res = run_bass_kernel_spmd(nc, in_maps, core_ids=list(range(len(in_maps))),
                               trace=trace)
    out = np.stack([r["out"] for r in res.results], axis=0).astype(np.float32)
    return out, res


def kernel(**inputs):
    out, _ = run(inputs, trace=False)
    return out
